# revision 33
# baseline (speedup 1.0000x reference)
"""Trainium2 Bass kernel for nn_BaseAttention (B=2, N=2048, E=2048, H=16, D=128).

Sharding: 8 cores; core c handles batch b=c//4, head-group hg=c%4 (4 heads).
Each core computes q/k/v projections for its heads, causal flash-style
attention, and a partial out-projection (contraction over its 512 head dims).
Host sums the 4 partial outputs per batch (tensor-parallel unshard).

Projections and out-projection run as float32r (full PE rate at free dim >=
256). q/k spill to DRAM as bf16, so QK^T and A@V are bf16 matmuls. exp runs
on ScalarE straight out of PSUM (only the causally-valid region); softmax
denominators use a DVE pairwise tree + ones-matmul partition reduction, a
K=1 broadcast matmul, and the fast approximate reciprocal.
"""

import os
import sys
import time

sys.path.insert(0, "/opt/trn_rl_repo")

PHASES = {"proj", "attn", "norm", "oproj"}

import numpy as np
import ml_dtypes

import concourse.bass as bass
import concourse.mybir as mybir
import concourse.tile as tile
from concourse import bacc
from concourse.bass_utils import run_bass_kernel_spmd

B, N, E, H = 2, 2048, 2048, 16
D = E // H            # 128
HPC = 4               # heads per core
DC = HPC * D          # 512 head dims per core
NCORES = 8
P = 128
NCH = N // 512        # 4 n-chunks of 512
ET = E // P           # 16 e-tiles of 128

F32 = mybir.dt.float32
F32R = mybir.dt.float32r
BF16 = mybir.dt.bfloat16


def build_nc():
    nc = bacc.Bacc("TRN2", target_bir_lowering=False, debug=False,
                   num_devices=NCORES)

    xT = nc.dram_tensor("xT", [E, N], FP16, kind="ExternalInput")
    wqT = nc.dram_tensor("wqT", [E, DC], FP16, kind="ExternalInput")
    wkT = nc.dram_tensor("wkT", [E, DC], FP16, kind="ExternalInput")
    wvT = nc.dram_tensor("wvT", [E, DC], FP16, kind="ExternalInput")
    woT = nc.dram_tensor("woT", [DC, E], FP16, kind="ExternalInput")
    maskin = nc.dram_tensor("maskin", [P, 4, 512], BF16, kind="ExternalInput")
    onesb_in = nc.dram_tensor("onesb", [P, 1], BF16, kind="ExternalInput")
    ones1_in = nc.dram_tensor("ones1", [1, P], F32R, kind="ExternalInput")
    out = nc.dram_tensor("out", [N, E], F32, kind="ExternalOutput")

    xT_r = xT.ap().rearrange("(eo p) n -> p eo n", p=P)      # [128,16,2048]
    wqT_r = wqT.ap().rearrange("(eo p) d -> p eo d", p=P)    # [128,16,512]
    wkT_r = wkT.ap().rearrange("(eo p) d -> p eo d", p=P)
    wvT_r = wvT.ap().rearrange("(eo p) d -> p eo d", p=P)
    woT_r = woT.ap().rearrange("(t p) e -> p t e", p=P)      # [128,4,2048]

    with tile.TileContext(nc) as tc:
        # ---------------- constants + spill tensors ----------------
        consts = tc.alloc_tile_pool(name="consts", bufs=1)
        _longlived = [consts]
        ones_bf = consts.tile([P, 1], BF16)
        nc.sync.dma_start(out=ones_bf, in_=onesb_in.ap())
        ones1 = consts.tile([1, P], F32R)
        nc.sync.dma_start(out=ones1, in_=ones1_in.ap())
        mask_sb = consts.tile([P, 4, 512], BF16)
        nc.sync.dma_start(out=mask_sb, in_=maskin.ap())
        # prefire the Exp table load so it overlaps the input DMA head
        dummy = consts.tile([1, 8], F32)
        nc.vector.memset(dummy, 0.0)
        nc.scalar.activation(out=dummy, in_=dummy,
                             func=mybir.ActivationFunctionType.Exp)

        h0pool = tc.alloc_tile_pool(name="h0pool", bufs=1)
        _longlived.append(h0pool)
        qh0 = h0pool.tile([P, N], BF16)
        kh0 = h0pool.tile([P, N], BF16)
        vh0 = h0pool.tile([P, N // P, D], BF16)

        dram = tc.alloc_tile_pool(name="dram", bufs=1, space="DRAM")
        _longlived.append(dram)
        qT_d = dram.tile([DC, N], BF16)              # [512, 2048]
        kT_d = dram.tile([DC, N], BF16)
        v_d = dram.tile([P, HPC, N // P, D], BF16)   # [128, 4, 16, 128]
        qT_dr = qT_d.rearrange("(t p) n -> p t n", p=P)  # [128, 4, 2048]
        kT_dr = kT_d.rearrange("(t p) n -> p t n", p=P)

        # ---------------- phase 1: q/k/v projections ----------------
        if "proj" in PHASES:
         with (
            tc.tile_pool(name="wpool", bufs=1) as wpool,
            tc.tile_pool(name="xpool", bufs=2) as xpool,
            tc.tile_pool(name="pj_ps", bufs=4, space="PSUM") as pj_ps,
            tc.tile_pool(name="pj_ev", bufs=3) as pj_ev,
         ):
            wq_sb = wpool.tile([P, ET, DC], FP16)
            wk_sb = wpool.tile([P, ET, DC], FP16)
            wv_sb = wpool.tile([P, ET, DC], FP16)
            x_tiles = [None] * NCH

            def load_x(nch):
                t = xpool.tile([P, ET, 512], FP16, tag="xchunk",
                               name=f"x_sb{nch}")
                nc.sync.dma_start(
                    out=t, in_=xT_r[:, :, nch * 512:(nch + 1) * 512])
                x_tiles[nch] = t

            # interleaved preload in 1MB pieces: PE starts after the first
            # wq piece + x0 piece; DMA then feeds just-in-time
            QT = ET // 4
            x0 = xpool.tile([P, ET, 512], FP16, tag="xchunk", name="x_sb0")
            x_tiles[0] = x0
            # extra-fine first pieces so the first matmul starts earlier
            for g2, (a, b) in enumerate(((0, 1), (1, 2), (2, 4))):
                gs = slice(a, b)
                nc.sync.dma_start(out=wq_sb[:, gs, :], in_=wqT_r[:, gs, :])
                nc.sync.dma_start(out=x0[:, gs, :],
                                  in_=xT_r[:, gs, 0:512])
            for g in range(1, 4):
                gs = slice(g * QT, (g + 1) * QT)
                nc.sync.dma_start(out=wq_sb[:, gs, :], in_=wqT_r[:, gs, :])
                nc.sync.dma_start(out=x0[:, gs, :],
                                  in_=xT_r[:, gs, 0:512])
            for g in range(4):
                gs = slice(g * QT, (g + 1) * QT)
                nc.sync.dma_start(out=wk_sb[:, gs, :], in_=wkT_r[:, gs, :])
            load_x(1)
            HF = ET // 2
            nc.sync.dma_start(out=wv_sb[:, :HF, :], in_=wvT_r[:, :HF, :])
            nc.sync.dma_start(out=wv_sb[:, HF:, :], in_=wvT_r[:, HF:, :])

            for nch in range(NCH):
                x_sb = x_tiles[nch]
                nsl = slice(nch * 512, (nch + 1) * 512)

                # qT / kT: psum[dq_tile 128, 512 n] = sum_e W[e, dq] x[e, n]
                for w_sb, dst in ((wq_sb, qT_dr), (wk_sb, kT_dr)):
                    stage = pj_ev.tile([P, HPC, 512], BF16, tag="qkev")
                    for t in range(HPC):
                        ps = pj_ps.tile([P, 512], F32, tag="pjps")
                        for et in range(ET):
                            nc.tensor.matmul(
                                ps,
                                lhsT=w_sb[:, et, t * P:(t + 1) * P],
                                rhs=x_sb[:, et, :],
                                start=(et == 0), stop=(et == ET - 1),
                            )
                        nc.any.tensor_copy(out=stage[:, t, :], in_=ps)
                    nc.sync.dma_start(out=dst[:, :, nsl], in_=stage)

                # v: psum[n_block 128, 512 dv] = sum_e x[e, n] Wv[e, dv]
                for nb in range(4):
                    ps = pj_ps.tile([P, 512], F32, tag="pjps")
                    for et in range(ET):
                        nc.tensor.matmul(
                            ps,
                            lhsT=x_sb[:, et, nb * P:(nb + 1) * P],
                            rhs=wv_sb[:, et, :],
                            start=(et == 0), stop=(et == ET - 1),
                        )
                    vstage = pj_ev.tile([P, HPC, D], BF16, tag="vev")
                    nc.any.tensor_copy(
                        out=vstage, in_=ps.rearrange("p (h d) -> p h d", h=HPC))
                    nc.sync.dma_start(
                        out=v_d[:, :, nch * 4 + nb, :], in_=vstage)

                if "attn" in PHASES:
                    # pipeline head-0's attention inputs chunk by chunk
                    nc.sync.dma_start(out=qh0[:, nsl], in_=qT_dr[:, 0, nsl])
                    nc.sync.dma_start(out=kh0[:, nsl], in_=kT_dr[:, 0, nsl])
                    nc.sync.dma_start(
                        out=vh0[:, nch * 4:(nch + 1) * 4, :],
                        in_=v_d[:, 0, nch * 4:(nch + 1) * 4, :])

                if nch + 2 < NCH:
                    load_x(nch + 2)

        # ---------------- phase 2: attention ----------------
        outT_pool = tc.alloc_tile_pool(name="outT", bufs=1)
        _longlived.append(outT_pool)
        out_hT = outT_pool.tile([P, HPC, N], F32R)       # [128, 4, 2048]
        sinv = outT_pool.tile([1, HPC * NCH, 512], F32R)  # holds S per (h,ci)
        wo_pool = tc.alloc_tile_pool(name="wo_pool", bufs=1)
        _longlived.append(wo_pool)
        wo_sb = wo_pool.tile([P, HPC, E], F32R)
        if "oproj" in PHASES and "attn" not in PHASES:
            for t in range(HPC):
                nc.sync.dma_start(out=wo_sb[:, t, :], in_=woT_r[:, t, :])

        if "attn" in PHASES:
         with (
            tc.tile_pool(name="qkv_h", bufs=2) as qkv_h,
            tc.tile_pool(name="pt_pool", bufs=3) as pt_pool,
            tc.tile_pool(name="tr_pool", bufs=2) as tr_pool,
            tc.tile_pool(name="qk_ps", bufs=2, space="PSUM") as qk_ps,
            tc.tile_pool(name="av_ps", bufs=4, space="PSUM") as av_ps,
            tc.tile_pool(name="s_ps", bufs=2, space="PSUM") as s_ps,
         ):
            for h in range(HPC):
                if h == 0 and "proj" in PHASES:
                    qh, kh, vh = qh0, kh0, vh0
                else:
                    qh = qkv_h.tile([P, N], BF16, tag="qh")
                    kh = qkv_h.tile([P, N], BF16, tag="kh")
                    vh = qkv_h.tile([P, N // P, D], BF16, tag="vh")
                    nc.sync.dma_start(out=qh, in_=qT_dr[:, h, :])
                    nc.sync.dma_start(out=kh, in_=kT_dr[:, h, :])
                    nc.sync.dma_start(out=vh, in_=v_d[:, h, :, :])
                if "oproj" in PHASES:
                    nc.sync.dma_start(out=wo_sb[:, h, :], in_=woT_r[:, h, :])

                for ci in range(NCH):
                    BJ = 4 * (ci + 1)
                    pt = pt_pool.tile([P, ET, 512], BF16, tag="pt")
                    if h == 0:
                        # first use of each slice range of the rotating pool:
                        # clear the regions partial-exp never writes so the
                        # mask multiply sees finite values, not NaN garbage
                        nc.vector.memset(pt[:, BJ - 2, :256], 0.0)
                        nc.vector.memset(pt[:, BJ - 1, :384], 0.0)
                    # scores^T tiles [j_block, i_chunk] + exp (2 tiles/ACT op)
                    for bjp in range(BJ // 2):
                        ps = qk_ps.tile([P, 2, 512], F32, tag="qkps")
                        last_pair = (bjp == BJ // 2 - 1)
                        for u in range(2):
                            bj = 2 * bjp + u
                            nc.tensor.matmul(
                                ps[:, u, :],
                                lhsT=kh[:, bj * P:(bj + 1) * P],
                                rhs=qh[:, ci * 512:(ci + 1) * 512],
                                start=True, stop=True,
                            )
                        if last_pair:
                            # diagonal blocks r=256,384: only cols >= r valid
                            nc.scalar.activation(
                                out=pt[:, 2 * bjp, 256:], in_=ps[:, 0, 256:],
                                func=mybir.ActivationFunctionType.Exp,
                            )
                            nc.scalar.activation(
                                out=pt[:, 2 * bjp + 1, 384:], in_=ps[:, 1, 384:],
                                func=mybir.ActivationFunctionType.Exp,
                            )
                        else:
                            nc.scalar.activation(
                                out=pt[:, 2 * bjp:2 * bjp + 2, :], in_=ps,
                                func=mybir.ActivationFunctionType.Exp,
                            )
                    # causal masks on the diagonal blocks (bj = BJ-4 .. BJ-1)
                    # full-tile: the mask's zero prefix also clears regions
                    # exp never wrote (stale finite values from pool reuse)
                    for rr in range(4):
                        bj = BJ - 4 + rr
                        nc.vector.tensor_mul(
                            out=pt[:, bj, :], in0=pt[:, bj, :],
                            in1=mask_sb[:, rr, :])

                    # S: pairwise-tree reduce over BJ (first pass on the
                    # otherwise-idle GPSIMD), then ones-matmul partition sum
                    trt = tr_pool.tile([P, ET // 2, 512], BF16, tag="trt")
                    half = BJ // 2
                    nc.vector.tensor_add(
                        out=trt[:, :half, :], in0=pt[:, :half, :],
                        in1=pt[:, half:BJ, :])
                    n_live = half
                    while n_live > 1:
                        k2 = n_live // 2
                        nc.vector.tensor_add(
                            out=trt[:, :k2, :], in0=trt[:, :k2, :],
                            in1=trt[:, k2:2 * k2, :])
                        if n_live % 2:
                            nc.vector.tensor_add(
                                out=trt[:, :1, :], in0=trt[:, :1, :],
                                in1=trt[:, 2 * k2:2 * k2 + 1, :])
                        n_live = k2
                    sp = s_ps.tile([1, 512], F32, tag="sps")
                    nc.tensor.matmul(sp, lhsT=ones_bf, rhs=trt[:, 0, :],
                                     start=True, stop=True)
                    nc.scalar.copy(
                        out=sinv[0:1, h * NCH + ci, :], in_=sp)

                    # A @ V (accumulate over j blocks)
                    avp = av_ps.tile([P, 512], F32, tag="avps")
                    for bj in range(BJ):
                        nc.tensor.matmul(
                            avp,
                            lhsT=vh[:, bj, :],
                            rhs=pt[:, bj, :],
                            start=(bj == 0), stop=(bj == BJ - 1),
                        )
                    nc.scalar.copy(
                        out=out_hT[:, h, ci * 512:(ci + 1) * 512], in_=avp)

        # ---------------- phase 3: normalize ----------------
        if "norm" in PHASES:
         with (
            tc.tile_pool(name="bc_ps", bufs=2, space="PSUM") as bc_ps,
            tc.tile_pool(name="bc_sb", bufs=2) as bc_sb,
         ):
            for h in range(HPC):
                for ci in range(NCH):
                    bp = bc_ps.tile([P, 512], F32, tag="bcps")
                    nc.tensor.matmul(bp, lhsT=ones1,
                                     rhs=sinv[0:1, h * NCH + ci, :],
                                     start=True, stop=True)
                    rp = bc_sb.tile([P, 512], F32, tag="rp")
                    nc.vector.reciprocal_approx_fast(out=rp, in_=bp)
                    sl = out_hT[:, h, ci * 512:(ci + 1) * 512]
                    nc.vector.tensor_mul(out=sl, in0=sl, in1=rp)

        # ---------------- phase 4: out projection (partial) ----------------
        if "oproj" in PHASES:
         with (
            tc.tile_pool(name="op_ps", bufs=4, space="PSUM") as op_ps,
            tc.tile_pool(name="op_ev", bufs=3) as op_ev,
         ):
            for nb in range(N // P):
                ostage = op_ev.tile([P, NCH, 512], F32, tag="opev")
                for ec in range(NCH):
                    ps = op_ps.tile([P, 512], F32, tag="opps")
                    for t in range(HPC):
                        nc.tensor.matmul(
                            ps,
                            lhsT=out_hT[:, t, nb * P:(nb + 1) * P],
                            rhs=wo_sb[:, t, ec * 512:(ec + 1) * 512],
                            start=(t == 0), stop=(t == HPC - 1),
                        )
                    nc.any.tensor_copy(out=ostage[:, ec, :], in_=ps)
                nc.sync.dma_start(
                    out=out.ap()[nb * P:(nb + 1) * P, :], in_=ostage)

        for _pl in reversed(_longlived):
            _pl.release()

    nc.compile()
    return nc


def make_in_maps(x, Wq, Wkv, Wout):
    x = np.asarray(x, dtype=np.float32)
    Wq = np.asarray(Wq, dtype=np.float32)
    Wkv = np.asarray(Wkv, dtype=np.float32)
    Wout = np.asarray(Wout, dtype=np.float32)
    scale = np.float32(D ** -0.5)

    # causal masks for the 4 diagonal offsets
    jj = np.arange(P)[:, None]
    ii = np.arange(512)[None, :]
    mask = np.zeros((P, 4, 512), dtype=ml_dtypes.bfloat16)
    for rr in range(4):
        mask[:, rr, :] = (ii >= jj + rr * P).astype(ml_dtypes.bfloat16)

    xT = [np.ascontiguousarray(x[b].T).astype(np.float16) for b in range(B)]
    in_maps = []
    for c in range(NCORES):
        b, hg = divmod(c, 4)
        sl = slice(hg * DC, (hg + 1) * DC)
        in_maps.append({
            "xT": xT[b],
            "wqT": (np.ascontiguousarray(Wq[sl, :].T) * scale).astype(np.float16),
            "wkT": np.ascontiguousarray(Wkv[sl, :].T).astype(np.float16),
            "wvT": np.ascontiguousarray(Wkv[E + sl.start:E + sl.stop, :].T).astype(np.float16),
            "woT": np.ascontiguousarray(Wout[:, sl].T).astype(np.float16),
            "maskin": mask,
            "onesb": np.ones((P, 1), dtype=ml_dtypes.bfloat16),
            "ones1": np.ones((1, P), dtype=np.float32),
        })
    return in_maps


_NC_CACHE = []


def _get_nc():
    if not _NC_CACHE:
        _NC_CACHE.append(build_nc())
    return _NC_CACHE[0]


def _run(in_maps):
    nc = _get_nc()
    return run_bass_kernel_spmd(nc, in_maps, core_ids=list(range(NCORES)))


def kernel(x, Wq, Wkv, Wout):
    in_maps = make_in_maps(x, Wq, Wkv, Wout)
    res = _run(in_maps)
    out = np.zeros((B, N, E), dtype=np.float32)
    for c in range(NCORES):
        out[c // 4] += res.results[c]["out"]
    return out


if __name__ == "__main__":
    t0 = time.time()
    _get_nc()
    print(f"build+compile: {time.time() - t0:.1f}s")
